# revision 24
# baseline (speedup 1.0000x reference)
"""Deep Richardson-Lucy deconvolution on 8 Trainium2 NeuronCores.

Strategy (per core, data-parallel batch shard of 512 rows):
- Everything lives in SBUF in a TRANSPOSED fp16 layout: [l on partitions
  (64 blocks of 128), batch on the free dim (512)]. The transpose/cast of
  m happens on the HOST (same fp16 rounding as on-chip), and the output
  un-transpose/fp32-cast also happens on the host, so the device does
  zero staging work: 4 chunked input DMAs, 4 chunked output DMAs.
- conv1d(K=31, zero-pad) == banded-Toeplitz matmul per 128-l block.
  conv1 (input s): center [128,128] matmul + ONE combined-halo matmul
  against an H1 ring tile filled by bulk same-partition SBUF->SBUF DMAs
  (rows 0:32 = next block's rows 0:32, rows 96:128 = prev block's rows
  96:128, middle rows permanently zero x zero weights).
  conv2 (input ratio): center + two 32-row halo matmuls reading the
  ratio tiles' partition subranges directly; the two halos land in
  disjoint PE quadrants (tile_position) and stream concurrently.
- Elementwise ops run on PAIRS of blocks (2-bank PSUM tiles, free size
  1024) to amortize per-instruction overhead and halve semaphore waits:
  r = ACT.Reciprocal(psum_pair + EPS); ratio = m * r (DVE fp16 2x);
  s *= (psum_pair + EPS) alternating DVE-stt / ACT-copy+DVE-mult.
- Deep software pipelining (ratio 2 pairs ahead, H1 chunks ~6 blocks
  ahead) keeps the PE 100% busy at full stream rate.
"""
import hashlib
import numpy as np

EPS = 1e-6
P = 128
KTAPS = 31
PAD = 15
B_FULL, L = 4096, 8192
N_CORES = 8
BC = B_FULL // N_CORES          # 512 batch rows per core
NT = L // P                     # 64 l-blocks
NP = NT // 2                    # 32 block pairs
NITER = 10

_cache = {}


def _build_toeplitz(psf):
    Wc = np.zeros((P, P), dtype=np.float64)
    j = np.arange(P)[:, None]
    i = np.arange(P)[None, :]
    k = j - i + PAD
    m = (k >= 0) & (k < KTAPS)
    Wc[m] = psf[k[m]]
    WL = np.zeros((32, 32), dtype=np.float64)   # rhs = prev block parts [96,128)
    jj = np.arange(32)[:, None]
    ii = np.arange(32)[None, :]
    k = (96 + jj - 128) - ii + PAD
    m = (k >= 0) & (k < KTAPS)
    WL[m] = psf[k[m]]
    WR = np.zeros((32, 32), dtype=np.float64)   # rhs = next block parts [0,32)
    k = (jj + 128) - (96 + ii) + PAD
    m = (k >= 0) & (k < KTAPS)
    WR[m] = psf[k[m]]
    return Wc, WL, WR


def _wpack(psf):
    """[P, 256] fp16: cols 0:128 = center Toeplitz; cols 128:256 = halo
    weight usable BOTH as one combined [128,128] matmul (against an H-pack
    tile whose rows 0:32 = next-block rows 0:32, rows 96:128 = prev-block
    rows 96:128, middle zero) AND as two separate 32-row matmuls
    (w[0:32,224:256]=WR -> out 96:128; w[96:128,128:160]=WL -> out 0:32)."""
    Wc, WL, WR = _build_toeplitz(psf)
    w = np.zeros((P, 256), dtype=np.float16)
    w[:, 0:128] = Wc
    w[0:32, 128 + 96:128 + 128] = WR
    w[96:128, 128 + 0:128 + 32] = WL
    return w


def _r0pack(psf64):
    """r0[p, t] = 1 / (conv1d(0.5*ones, psf)[128t+p] + EPS)."""
    ones = np.full((1, L), 0.5, dtype=np.float64)
    xp = np.pad(ones, ((0, 0), (PAD, PAD)))
    sc = np.zeros((1, L), dtype=np.float64)
    for k in range(KTAPS):
        sc += xp[:, k:k + L] * psf64[k]
    r = 1.0 / (sc[0] + EPS)
    return r.reshape(NT, P).T.astype(np.float32)


def _build(psf64, alpha64):
    import concourse.bass as bass
    import concourse.tile as tile
    from concourse import mybir
    import bass_rust

    F32 = mybir.dt.float32
    F16 = mybir.dt.float16

    class SafeTC(tile.TileContext):
        # this walrus build rejects >1 sync wait per CTRL-class instruction
        def _drain_and_barrier(self, tick_clock, wait_clock):
            gc = tick_clock.global_clock
            for i in range(len(gc)):
                if gc[i] > 0:
                    di = self.nc.sync.drain()
                    pc = bass_rust.VectorClock()
                    pc.require_at_least(i, gc[i])
                    wait_clock.add_sem_waits(di.ins, bass_rust.ScopedClock({None: pc}))
            self.nc.all_engine_barrier()
            popped = self.nc._tile_sem_poison_stack.pop()
            assert popped is self._sem_poison
            self.nc.clear_and_free_semaphores(list(self.sems.allocated().values()))
            self.nc.all_engine_barrier()

    def split_multi_waits(nc, max_waits=1):
        n_fixed = 0
        uid = [0]
        for f in nc.m.functions:
            for bb in f.blocks:
                out = []
                changed = False
                for inst in bb.instructions:
                    si = inst.sync_info
                    if si is not None:
                        sems = [w for w in si.on_wait
                                if str(getattr(w, "sync_type", "")) == "semaphore"]
                        other = [w for w in si.on_wait if w not in sems]
                        if len(sems) > max_waits:
                            keep = sems[-max_waits:]
                            for w in sems[:-max_waits]:
                                nop = mybir.InstNoOp(
                                    name=f"waitsplit_{uid[0]}", ins=[], outs=[])
                                uid[0] += 1
                                nop.engine = inst.engine
                                nop.sync_info = mybir.SyncInfo(
                                    on_wait=[w], on_update=[])
                                out.append(nop)
                            inst.sync_info = mybir.SyncInfo(
                                on_wait=other + keep,
                                on_update=list(si.on_update))
                            n_fixed += 1
                            changed = True
                    out.append(inst)
                if changed:
                    try:
                        bb.instructions = out
                    except Exception:
                        bb.instructions.clear()
                        bb.instructions.extend(out)
        return n_fixed

    def act_raw(nc, out, in_, func, bias=0.0, scale=1.0):
        eng = nc.scalar
        ins = [eng.lower_ap(in_),
               mybir.ImmediateValue(dtype=F32, value=float(bias)),
               mybir.ImmediateValue(dtype=F32, value=float(scale)),
               mybir.ImmediateValue(dtype=F32, value=0.0)]
        return eng.add_instruction(mybir.InstActivation(
            name=nc.get_next_instruction_name(), func=func, ins=ins,
            outs=[eng.lower_ap(out)]))

    alpha_is_one = bool(np.all(alpha64 == 1.0))

    nc = bass.Bass("TRN2", target_bir_lowering=False, debug=False,
                   num_devices=N_CORES)
    mT_in = nc.dram_tensor("mT", [P, NT, BC], F16, kind="ExternalInput")
    w1_in = nc.dram_tensor("w1", [P, 256], F16, kind="ExternalInput")
    w2_in = nc.dram_tensor("w2", [P, 256], F16, kind="ExternalInput")
    r0_in = nc.dram_tensor("r0", [P, NT], F32, kind="ExternalInput")
    y_out = nc.dram_tensor("y", [P, NT, BC], F16, kind="ExternalOutput")

    Rec = mybir.ActivationFunctionType.Reciprocal
    Ln = mybir.ActivationFunctionType.Ln
    Exp = mybir.ActivationFunctionType.Exp

    RING = 16                 # H1 ring columns
    CH = 4                    # blocks per H1 bulk-copy chunk

    def conv1_block(psum, w, s, h1ring, t):
        """Center matmul + ONE combined-halo matmul against the H1 ring
        column (rows 0:32 = s rows 0:32 of block t+1, rows 96:128 =
        s rows 96:128 of block t-1, middle rows permanently zero)."""
        r = t % RING
        nc.tensor.matmul(psum, w[:, 0:128], s[:, t, :],
                         start=True, stop=False)
        if t == 0:
            nc.tensor.matmul(psum, w[0:32, 128:256], h1ring[0:32, r, :],
                             start=False, stop=True)
        elif t == NT - 1:
            nc.tensor.matmul(psum, w[96:128, 128:256],
                             h1ring[96:128, r, :],
                             start=False, stop=True, tile_position=(96, 0))
        else:
            nc.tensor.matmul(psum, w[:, 128:256], h1ring[:, r, :],
                             start=False, stop=True)

    def conv2_block(psum, w, rat, h2r, t):
        """Center + ONE combined-halo matmul against the H2 ring column
        (DMA-packed from the ratio quad tiles, like conv1's H1)."""
        r = t % RING
        nc.tensor.matmul(psum, w[:, 0:128], rat(t),
                         start=True, stop=False)
        if t == 0:
            nc.tensor.matmul(psum, w[0:32, 128:256], h2r[0:32, r, :],
                             start=False, stop=True)
        elif t == NT - 1:
            nc.tensor.matmul(psum, w[96:128, 128:256],
                             h2r[96:128, r, :],
                             start=False, stop=True, tile_position=(96, 0))
        else:
            nc.tensor.matmul(psum, w[:, 128:256], h2r[:, r, :],
                             start=False, stop=True)

    def h1_chunk(h1ring, s, c):
        """Bulk-copy H1 ring slots for blocks [4c, 4c+4): same-partition
        SBUF->SBUF DMAs (one per side), contiguous 4KB per partition."""
        t0, t1 = CH * c, min(CH * (c + 1), NT)
        r0c = (CH * c) % RING
        lo, hi = t0 + 1, min(t1 + 1, NT)
        if hi > lo:
            nc.sync.dma_start(h1ring[0:32, r0c:r0c + (hi - lo), :],
                              s[0:32, lo:hi, :])
        lo, hi = max(t0 - 1, 0), t1 - 1
        if hi > lo:
            rb = r0c if t0 > 0 else r0c + 1
            nc.sync.dma_start(h1ring[96:128, rb:rb + (hi - lo), :],
                              s[96:128, lo:hi, :])

    with SafeTC(nc) as tc:
        with tc.tile_pool(name="wpool", bufs=1) as wpool, \
             tc.tile_pool(name="mpool", bufs=1) as mpool, \
             tc.tile_pool(name="spool", bufs=1) as spool:
            w1 = wpool.tile([P, 256], F16)
            nc.sync.dma_start(w1[:], w1_in[:])
            w2 = wpool.tile([P, 256], F16)
            nc.sync.dma_start(w2[:], w2_in[:])
            r0 = wpool.tile([P, NT], F32)
            nc.sync.dma_start(r0[:], r0_in[:])
            mT = mpool.tile([P, NT, BC], F16)
            # chunked loads across 8 DMA queues: iter0 starts on chunk 0
            # quickly and the full tile lands in ~8us instead of ~25us.
            for q in range(8):
                nc.sync.dma_start(mT[:, q * 8:(q + 1) * 8, :],
                                  mT_in[:, q * 8:(q + 1) * 8, :])
            s = spool.tile([P, NT, BC], F16)
            # quarter memsets on the idle Pool engine: startup overlaps the
            # mT chunk loads, and early updates only wait on quarter 0.
            for q in range(4):
                nc.gpsimd.memset(s[:, q * 16:(q + 1) * 16, :], 0.5)
            h1r = spool.tile([P, RING, BC], F16)
            nc.gpsimd.memset(h1r[:], 0.0)
            h2r = spool.tile([P, RING, BC], F16)
            nc.gpsimd.memset(h2r[:], 0.0)

            # ---- RL iterations (block pairs u=2j; ratio in QUAD tiles so
            # conv2's matmuls depend on 3x fewer producer semaphores) ----
            with tc.tile_pool(name="ratio", bufs=5) as rpool, \
                 tc.tile_pool(name="rtile", bufs=4) as rtp, \
                 tc.tile_pool(name="psum", bufs=4, space="PSUM") as pp:
                for it in range(NITER):
                    NQ = NT // 4
                    ratio_quads = [None] * NQ
                    rt_quads = [None] * NQ

                    def rat(t):
                        return ratio_quads[t // 4][:, t % 4, :]

                    def _conv1_recip_pair(j):
                        # conv1 + reciprocal for pair j, written into the
                        # matching half of the quad rt tile.
                        u = 2 * j
                        q4 = j // 2
                        if j % 2 == 0:
                            rtq = rtp.tile([P, 4, BC], F16, tag="rtq")
                            rt_quads[q4] = rtq
                        ps = pp.tile([P, 2, BC], mybir.dt.float32, tag="ps")
                        conv1_block(ps[:, 0, :], w1, s, h1r, u)
                        conv1_block(ps[:, 1, :], w1, s, h1r, u + 1)
                        half = (j % 2) * 2
                        act_raw(nc, rt_quads[q4][:, half:half + 2, :],
                                ps[:], Rec, bias=EPS)

                    def h2_chunk(c):
                        """DMA-pack H2 ring slots for blocks [4c, 4c+4)
                        from the ratio quad tiles (same-partition copies)."""
                        r = (4 * c) % RING
                        rq = ratio_quads
                        nc.sync.dma_start(h2r[0:32, r:r + 3, :],
                                          rq[c][0:32, 1:4, :])
                        if c < NQ - 1:
                            nc.sync.dma_start(h2r[0:32, r + 3:r + 4, :],
                                              rq[c + 1][0:32, 0:1, :])
                        if c > 0:
                            nc.sync.dma_start(h2r[96:128, r:r + 1, :],
                                              rq[c - 1][96:128, 3:4, :])
                        nc.sync.dma_start(h2r[96:128, r + 1:r + 4, :],
                                          rq[c][96:128, 0:3, :])

                    def _ratio_quad(q4):
                        u = 4 * q4
                        ra = rpool.tile([P, 4, BC], F16, tag="ra")
                        if it == 0:
                            # s == 0.5 everywhere: conv(s)+EPS is a per-l
                            # constant; r0 = 1/that, precomputed on host.
                            for k in range(4):
                                nc.vector.tensor_scalar(
                                    out=ra[:, k, :], in0=mT[:, u + k, :],
                                    scalar1=r0[:, u + k:u + k + 1],
                                    scalar2=None,
                                    op0=mybir.AluOpType.mult)
                        else:
                            # ONE fp16 2x DVE multiply per 4 blocks
                            nc.vector.tensor_mul(ra[:], mT[:, u:u + 4, :],
                                                 rt_quads[q4][:])
                        ratio_quads[q4] = ra

                    def _conv2_update_pair(j):
                        u = 2 * j
                        ps = pp.tile([P, 2, BC], mybir.dt.float32, tag="ps")
                        conv2_block(ps[:, 0, :], w2, rat, h2r, u)
                        conv2_block(ps[:, 1, :], w2, rat, h2r, u + 1)
                        if alpha_is_one:
                            if j % 2 == 0:
                                # DVE fused: s = (psum + EPS) * s, PSUM 1x
                                nc.vector.scalar_tensor_tensor(
                                    out=s[:, u:u + 2, :], in0=ps[:],
                                    scalar=EPS, in1=s[:, u:u + 2, :],
                                    op0=mybir.AluOpType.add,
                                    op1=mybir.AluOpType.mult)
                            else:
                                # ACT evacuates PSUM (+EPS), DVE fp16 2x mul
                                cp = rtp.tile([P, 2, BC], F16, tag="cp")
                                act_raw(nc, cp[:], ps[:],
                                        mybir.ActivationFunctionType.Copy,
                                        bias=EPS)
                                nc.vector.tensor_mul(s[:, u:u + 2, :],
                                                     s[:, u:u + 2, :], cp[:])
                        else:
                            lg = rtp.tile([P, 2, BC], F32, tag="lg")
                            act_raw(nc, lg[:], ps[:], Ln, bias=EPS)
                            cp = rtp.tile([P, 2, BC], F16, tag="cp")
                            act_raw(nc, cp[:], lg[:], Exp,
                                    scale=float(alpha64[it]))
                            nc.vector.tensor_mul(s[:, u:u + 2, :],
                                                 s[:, u:u + 2, :], cp[:])
                        # stream the finished s quarter out during iter 9
                        if it == NITER - 1 and (j + 1) % 8 == 0:
                            q = j // 8
                            nc.sync.dma_start(
                                y_out[:, q * 16:(q + 1) * 16, :],
                                s[:, q * 16:(q + 1) * 16, :])

    # software-pipelined emission (per quad step qq):
                    # H1 chunks ~1 quad ahead of conv1, ratio quads 1 quad
                    # ahead of conv2.
                    def _produce_quad(q4):
                        if it > 0:
                            c = q4 + 2              # H1 chunk, 2 quads ahead
                            if c < NT // CH:
                                h1_chunk(h1r, s, c)
                            _conv1_recip_pair(2 * q4)
                            _conv1_recip_pair(2 * q4 + 1)
                        _ratio_quad(q4)

                    if it > 0:
                        h1_chunk(h1r, s, 0)
                        h1_chunk(h1r, s, 1)
                    _produce_quad(0)
                    _produce_quad(1)
                    h2_chunk(0)
                    for qq in range(NQ):
                        if qq + 2 < NQ:
                            _produce_quad(qq + 2)
                        if qq + 1 < NQ:
                            h2_chunk(qq + 1)
                        _conv2_update_pair(2 * qq)
                        _conv2_update_pair(2 * qq + 1)

    split_multi_waits(nc)
    return nc


def _make_in_maps(m, psf, alpha):
    m = np.asarray(m)
    psf64 = np.asarray(psf, dtype=np.float64)
    w1 = _wpack(psf64)
    w2 = _wpack(psf64[::-1])
    r0 = _r0pack(psf64)
    in_maps = []
    for c in range(N_CORES):
        mc = m[c * BC:(c + 1) * BC].astype(np.float16)      # [BC, L]
        mT = np.ascontiguousarray(
            mc.reshape(BC, NT, P).transpose(2, 1, 0))        # [P, NT, BC]
        in_maps.append({"mT": mT, "w1": w1, "w2": w2, "r0": r0})
    return in_maps


def kernel(m, psf, alpha):
    m = np.asarray(m)
    psf64 = np.asarray(psf, dtype=np.float64)
    alpha64 = np.asarray(alpha, dtype=np.float64)
    key = hashlib.sha256(
        psf64.tobytes() + alpha64.tobytes() + str(m.shape).encode()).hexdigest()
    if key not in _cache:
        _cache[key] = _build(psf64, alpha64)
    nc = _cache[key]

    from concourse.bass_utils import run_bass_kernel_spmd
    in_maps = _make_in_maps(m, psf, alpha)
    res = run_bass_kernel_spmd(nc, in_maps, core_ids=list(range(N_CORES)))
    outs = []
    for c in range(N_CORES):
        yT = res.results[c]["y"]                             # [P, NT, BC] fp16
        outs.append(np.asarray(yT).transpose(2, 1, 0).reshape(BC, L))
    return np.concatenate(outs, axis=0).astype(np.float32)


# revision 25
# speedup vs baseline: 1.3490x; 1.3490x over previous
"""Deep Richardson-Lucy deconvolution on 8 Trainium2 NeuronCores.

Strategy (per core, data-parallel batch shard of 512 rows):
- Everything lives in SBUF in a TRANSPOSED fp16 layout: [l on partitions
  (64 blocks of 128), batch on the free dim (512)]. The transpose/cast of
  m happens on the HOST (same fp16 rounding as on-chip), and the output
  un-transpose/fp32-cast also happens on the host, so the device does
  zero staging work: 4 chunked input DMAs, 4 chunked output DMAs.
- conv1d(K=31, zero-pad) == banded-Toeplitz matmul per 128-l block.
  conv1 (input s): center [128,128] matmul + ONE combined-halo matmul
  against an H1 ring tile filled by bulk same-partition SBUF->SBUF DMAs
  (rows 0:32 = next block's rows 0:32, rows 96:128 = prev block's rows
  96:128, middle rows permanently zero x zero weights).
  conv2 (input ratio): center + two 32-row halo matmuls reading the
  ratio tiles' partition subranges directly; the two halos land in
  disjoint PE quadrants (tile_position) and stream concurrently.
- Elementwise ops run on PAIRS of blocks (2-bank PSUM tiles, free size
  1024) to amortize per-instruction overhead and halve semaphore waits:
  r = ACT.Reciprocal(psum_pair + EPS); ratio = m * r (DVE fp16 2x);
  s *= (psum_pair + EPS) alternating DVE-stt / ACT-copy+DVE-mult.
- Deep software pipelining (ratio 2 pairs ahead, H1 chunks ~6 blocks
  ahead) keeps the PE 100% busy at full stream rate.
"""
import hashlib
import numpy as np

EPS = 1e-6
P = 128
KTAPS = 31
PAD = 15
B_FULL, L = 4096, 8192
N_CORES = 8
BC = B_FULL // N_CORES          # 512 batch rows per core
NT = L // P                     # 64 l-blocks
NP = NT // 2                    # 32 block pairs
NITER = 10

_cache = {}


def _build_toeplitz(psf):
    Wc = np.zeros((P, P), dtype=np.float64)
    j = np.arange(P)[:, None]
    i = np.arange(P)[None, :]
    k = j - i + PAD
    m = (k >= 0) & (k < KTAPS)
    Wc[m] = psf[k[m]]
    WL = np.zeros((32, 32), dtype=np.float64)   # rhs = prev block parts [96,128)
    jj = np.arange(32)[:, None]
    ii = np.arange(32)[None, :]
    k = (96 + jj - 128) - ii + PAD
    m = (k >= 0) & (k < KTAPS)
    WL[m] = psf[k[m]]
    WR = np.zeros((32, 32), dtype=np.float64)   # rhs = next block parts [0,32)
    k = (jj + 128) - (96 + ii) + PAD
    m = (k >= 0) & (k < KTAPS)
    WR[m] = psf[k[m]]
    return Wc, WL, WR


def _wpack(psf):
    """[P, 256] fp16: cols 0:128 = center Toeplitz; cols 128:256 = halo
    weight usable BOTH as one combined [128,128] matmul (against an H-pack
    tile whose rows 0:32 = next-block rows 0:32, rows 96:128 = prev-block
    rows 96:128, middle zero) AND as two separate 32-row matmuls
    (w[0:32,224:256]=WR -> out 96:128; w[96:128,128:160]=WL -> out 0:32)."""
    Wc, WL, WR = _build_toeplitz(psf)
    w = np.zeros((P, 256), dtype=np.float16)
    w[:, 0:128] = Wc
    w[0:32, 128 + 96:128 + 128] = WR
    w[96:128, 128 + 0:128 + 32] = WL
    return w


def _r0pack(psf64):
    """r0[p, t] = 1 / (conv1d(0.5*ones, psf)[128t+p] + EPS)."""
    ones = np.full((1, L), 0.5, dtype=np.float64)
    xp = np.pad(ones, ((0, 0), (PAD, PAD)))
    sc = np.zeros((1, L), dtype=np.float64)
    for k in range(KTAPS):
        sc += xp[:, k:k + L] * psf64[k]
    r = 1.0 / (sc[0] + EPS)
    return r.reshape(NT, P).T.astype(np.float32)


def _build(psf64, alpha64):
    import concourse.bass as bass
    import concourse.tile as tile
    from concourse import mybir
    import bass_rust

    F32 = mybir.dt.float32
    F16 = mybir.dt.float16

    class SafeTC(tile.TileContext):
        # this walrus build rejects >1 sync wait per CTRL-class instruction
        def _drain_and_barrier(self, tick_clock, wait_clock):
            gc = tick_clock.global_clock
            for i in range(len(gc)):
                if gc[i] > 0:
                    di = self.nc.sync.drain()
                    pc = bass_rust.VectorClock()
                    pc.require_at_least(i, gc[i])
                    wait_clock.add_sem_waits(di.ins, bass_rust.ScopedClock({None: pc}))
            self.nc.all_engine_barrier()
            popped = self.nc._tile_sem_poison_stack.pop()
            assert popped is self._sem_poison
            self.nc.clear_and_free_semaphores(list(self.sems.allocated().values()))
            self.nc.all_engine_barrier()

    def split_multi_waits(nc, max_waits=1):
        n_fixed = 0
        uid = [0]
        for f in nc.m.functions:
            for bb in f.blocks:
                out = []
                changed = False
                for inst in bb.instructions:
                    si = inst.sync_info
                    if si is not None:
                        sems = [w for w in si.on_wait
                                if str(getattr(w, "sync_type", "")) == "semaphore"]
                        other = [w for w in si.on_wait if w not in sems]
                        if len(sems) > max_waits:
                            keep = sems[-max_waits:]
                            for w in sems[:-max_waits]:
                                nop = mybir.InstNoOp(
                                    name=f"waitsplit_{uid[0]}", ins=[], outs=[])
                                uid[0] += 1
                                nop.engine = inst.engine
                                nop.sync_info = mybir.SyncInfo(
                                    on_wait=[w], on_update=[])
                                out.append(nop)
                            inst.sync_info = mybir.SyncInfo(
                                on_wait=other + keep,
                                on_update=list(si.on_update))
                            n_fixed += 1
                            changed = True
                    out.append(inst)
                if changed:
                    try:
                        bb.instructions = out
                    except Exception:
                        bb.instructions.clear()
                        bb.instructions.extend(out)
        return n_fixed

    def act_raw(nc, out, in_, func, bias=0.0, scale=1.0):
        eng = nc.scalar
        ins = [eng.lower_ap(in_),
               mybir.ImmediateValue(dtype=F32, value=float(bias)),
               mybir.ImmediateValue(dtype=F32, value=float(scale)),
               mybir.ImmediateValue(dtype=F32, value=0.0)]
        return eng.add_instruction(mybir.InstActivation(
            name=nc.get_next_instruction_name(), func=func, ins=ins,
            outs=[eng.lower_ap(out)]))

    alpha_is_one = bool(np.all(alpha64 == 1.0))

    nc = bass.Bass("TRN2", target_bir_lowering=False, debug=False,
                   num_devices=N_CORES)
    mT_in = nc.dram_tensor("mT", [P, NT, BC], F16, kind="ExternalInput")
    w1_in = nc.dram_tensor("w1", [P, 256], F16, kind="ExternalInput")
    w2_in = nc.dram_tensor("w2", [P, 256], F16, kind="ExternalInput")
    r0_in = nc.dram_tensor("r0", [P, NT], F32, kind="ExternalInput")
    y_out = nc.dram_tensor("y", [P, NT, BC], F16, kind="ExternalOutput")

    Rec = mybir.ActivationFunctionType.Reciprocal
    Ln = mybir.ActivationFunctionType.Ln
    Exp = mybir.ActivationFunctionType.Exp

    RING = 16                 # H1 ring columns
    CH = 4                    # blocks per H1 bulk-copy chunk

    def conv1_block(psum, w, s, h1ring, t):
        """Center matmul + ONE combined-halo matmul against the H1 ring
        column (rows 0:32 = s rows 0:32 of block t+1, rows 96:128 =
        s rows 96:128 of block t-1, middle rows permanently zero)."""
        r = t % RING
        nc.tensor.matmul(psum, w[:, 0:128], s[:, t, :],
                         start=True, stop=False)
        if t == 0:
            nc.tensor.matmul(psum, w[0:32, 128:256], h1ring[0:32, r, :],
                             start=False, stop=True)
        elif t == NT - 1:
            nc.tensor.matmul(psum, w[96:128, 128:256],
                             h1ring[96:128, r, :],
                             start=False, stop=True, tile_position=(96, 0))
        else:
            nc.tensor.matmul(psum, w[:, 128:256], h1ring[:, r, :],
                             start=False, stop=True)

    def conv2_block(psum, w, rat, t):
        """Center + two 32-row halo matmuls reading the ratio tiles'
        partition subranges directly; halos go to disjoint PE quadrants."""
        last = "R" if t < NT - 1 else "L"
        nc.tensor.matmul(psum, w[:, 0:128], rat(t),
                         start=True, stop=False)
        if t > 0:
            nc.tensor.matmul(psum[0:32, :], w[96:128, 128:160],
                             rat(t - 1)[96:128, :], start=False,
                             stop=(last == "L"), tile_position=(96, 0))
        if t < NT - 1:
            nc.tensor.matmul(psum[96:128, :], w[0:32, 224:256],
                             rat(t + 1)[0:32, :], start=False,
                             stop=(last == "R"), tile_position=(0, 96))

    def h1_chunk(h1ring, s, c):
        """Bulk-copy H1 ring slots for blocks [4c, 4c+4): same-partition
        SBUF->SBUF DMAs (one per side), contiguous 4KB per partition."""
        t0, t1 = CH * c, min(CH * (c + 1), NT)
        r0c = (CH * c) % RING
        lo, hi = t0 + 1, min(t1 + 1, NT)
        if hi > lo:
            nc.sync.dma_start(h1ring[0:32, r0c:r0c + (hi - lo), :],
                              s[0:32, lo:hi, :])
        lo, hi = max(t0 - 1, 0), t1 - 1
        if hi > lo:
            rb = r0c if t0 > 0 else r0c + 1
            nc.sync.dma_start(h1ring[96:128, rb:rb + (hi - lo), :],
                              s[96:128, lo:hi, :])

    with SafeTC(nc) as tc:
        with tc.tile_pool(name="wpool", bufs=1) as wpool, \
             tc.tile_pool(name="mpool", bufs=1) as mpool, \
             tc.tile_pool(name="spool", bufs=1) as spool:
            w1 = wpool.tile([P, 256], F16)
            nc.sync.dma_start(w1[:], w1_in[:])
            w2 = wpool.tile([P, 256], F16)
            nc.sync.dma_start(w2[:], w2_in[:])
            r0 = wpool.tile([P, NT], F32)
            nc.sync.dma_start(r0[:], r0_in[:])
            mT = mpool.tile([P, NT, BC], F16)
            # chunked loads across 8 DMA queues: iter0 starts on chunk 0
            # quickly and the full tile lands in ~8us instead of ~25us.
            for q in range(8):
                nc.sync.dma_start(mT[:, q * 8:(q + 1) * 8, :],
                                  mT_in[:, q * 8:(q + 1) * 8, :])
            s = spool.tile([P, NT, BC], F16)
            # quarter memsets on the idle Pool engine: startup overlaps the
            # mT chunk loads, and early updates only wait on quarter 0.
            for q in range(4):
                nc.gpsimd.memset(s[:, q * 16:(q + 1) * 16, :], 0.5)
            h1r = spool.tile([P, RING, BC], F16)
            nc.gpsimd.memset(h1r[:], 0.0)

            # ---- RL iterations (block pairs u=2j; ratio in QUAD tiles so
            # conv2's matmuls depend on 3x fewer producer semaphores) ----
            with tc.tile_pool(name="ratio", bufs=5) as rpool, \
                 tc.tile_pool(name="rtile", bufs=4) as rtp, \
                 tc.tile_pool(name="psum", bufs=4, space="PSUM") as pp:
                for it in range(NITER):
                    NQ = NT // 4
                    ratio_quads = [None] * NQ
                    rt_quads = [None] * NQ

                    def rat(t):
                        return ratio_quads[t // 4][:, t % 4, :]

                    def _conv1_recip_pair(j):
                        # conv1 + reciprocal for pair j, written into the
                        # matching half of the quad rt tile.
                        u = 2 * j
                        q4 = j // 2
                        if j % 2 == 0:
                            rtq = rtp.tile([P, 4, BC], F16, tag="rtq")
                            rt_quads[q4] = rtq
                        ps = pp.tile([P, 2, BC], mybir.dt.float32, tag="ps")
                        conv1_block(ps[:, 0, :], w1, s, h1r, u)
                        conv1_block(ps[:, 1, :], w1, s, h1r, u + 1)
                        half = (j % 2) * 2
                        act_raw(nc, rt_quads[q4][:, half:half + 2, :],
                                ps[:], Rec, bias=EPS)

                    def _ratio_quad(q4):
                        u = 4 * q4
                        ra = rpool.tile([P, 4, BC], F16, tag="ra")
                        if it == 0:
                            # s == 0.5 everywhere: conv(s)+EPS is a per-l
                            # constant; r0 = 1/that, precomputed on host.
                            for k in range(4):
                                nc.vector.tensor_scalar(
                                    out=ra[:, k, :], in0=mT[:, u + k, :],
                                    scalar1=r0[:, u + k:u + k + 1],
                                    scalar2=None,
                                    op0=mybir.AluOpType.mult)
                        else:
                            # ONE fp16 2x DVE multiply per 4 blocks
                            nc.vector.tensor_mul(ra[:], mT[:, u:u + 4, :],
                                                 rt_quads[q4][:])
                        ratio_quads[q4] = ra

                    def _conv2_update_pair(j):
                        u = 2 * j
                        ps = pp.tile([P, 2, BC], mybir.dt.float32, tag="ps")
                        conv2_block(ps[:, 0, :], w2, rat, u)
                        conv2_block(ps[:, 1, :], w2, rat, u + 1)
                        if alpha_is_one:
                            if j % 2 == 0:
                                # DVE fused: s = (psum + EPS) * s, PSUM 1x
                                nc.vector.scalar_tensor_tensor(
                                    out=s[:, u:u + 2, :], in0=ps[:],
                                    scalar=EPS, in1=s[:, u:u + 2, :],
                                    op0=mybir.AluOpType.add,
                                    op1=mybir.AluOpType.mult)
                            else:
                                # ACT evacuates PSUM (+EPS), DVE fp16 2x mul
                                cp = rtp.tile([P, 2, BC], F16, tag="cp")
                                act_raw(nc, cp[:], ps[:],
                                        mybir.ActivationFunctionType.Copy,
                                        bias=EPS)
                                nc.vector.tensor_mul(s[:, u:u + 2, :],
                                                     s[:, u:u + 2, :], cp[:])
                        else:
                            lg = rtp.tile([P, 2, BC], F32, tag="lg")
                            act_raw(nc, lg[:], ps[:], Ln, bias=EPS)
                            cp = rtp.tile([P, 2, BC], F16, tag="cp")
                            act_raw(nc, cp[:], lg[:], Exp,
                                    scale=float(alpha64[it]))
                            nc.vector.tensor_mul(s[:, u:u + 2, :],
                                                 s[:, u:u + 2, :], cp[:])
                        # stream the finished s quarter out during iter 9
                        if it == NITER - 1 and (j + 1) % 8 == 0:
                            q = j // 8
                            nc.sync.dma_start(
                                y_out[:, q * 16:(q + 1) * 16, :],
                                s[:, q * 16:(q + 1) * 16, :])

    # software-pipelined emission (per quad step qq):
                    # H1 chunks ~1 quad ahead of conv1, ratio quads 1 quad
                    # ahead of conv2.
                    def _produce_quad(q4):
                        if it > 0:
                            c = q4 + 2              # H1 chunk, 2 quads ahead
                            if c < NT // CH:
                                h1_chunk(h1r, s, c)
                            _conv1_recip_pair(2 * q4)
                            _conv1_recip_pair(2 * q4 + 1)
                        _ratio_quad(q4)

                    if it > 0:
                        h1_chunk(h1r, s, 0)
                        h1_chunk(h1r, s, 1)
                    _produce_quad(0)
                    _produce_quad(1)
                    for qq in range(NQ):
                        if qq + 2 < NQ:
                            _produce_quad(qq + 2)
                        _conv2_update_pair(2 * qq)
                        _conv2_update_pair(2 * qq + 1)

    split_multi_waits(nc)
    return nc


def _make_in_maps(m, psf, alpha):
    m = np.asarray(m)
    psf64 = np.asarray(psf, dtype=np.float64)
    w1 = _wpack(psf64)
    w2 = _wpack(psf64[::-1])
    r0 = _r0pack(psf64)
    in_maps = []
    for c in range(N_CORES):
        mc = m[c * BC:(c + 1) * BC].astype(np.float16)      # [BC, L]
        mT = np.ascontiguousarray(
            mc.reshape(BC, NT, P).transpose(2, 1, 0))        # [P, NT, BC]
        in_maps.append({"mT": mT, "w1": w1, "w2": w2, "r0": r0})
    return in_maps


def kernel(m, psf, alpha):
    m = np.asarray(m)
    psf64 = np.asarray(psf, dtype=np.float64)
    alpha64 = np.asarray(alpha, dtype=np.float64)
    key = hashlib.sha256(
        psf64.tobytes() + alpha64.tobytes() + str(m.shape).encode()).hexdigest()
    if key not in _cache:
        _cache[key] = _build(psf64, alpha64)
    nc = _cache[key]

    from concourse.bass_utils import run_bass_kernel_spmd
    in_maps = _make_in_maps(m, psf, alpha)
    res = run_bass_kernel_spmd(nc, in_maps, core_ids=list(range(N_CORES)))
    outs = []
    for c in range(N_CORES):
        yT = res.results[c]["y"]                             # [P, NT, BC] fp16
        outs.append(np.asarray(yT).transpose(2, 1, 0).reshape(BC, L))
    return np.concatenate(outs, axis=0).astype(np.float32)
